# revision 18
# baseline (speedup 1.0000x reference)
"""Trainium2 Bass kernel for per-head Llama GQA attention.

Model: H=16 q heads, HKV=4 kv heads, head_dim=128, L=2048, D=2048, B=1.
Per-head hidden streams and per-head outputs (no cross-head reduction), so
tensor-parallel over heads is embarrassingly parallel: core c owns q heads
{2c, 2c+1} and their kv head c//2.  No collectives.

v3: fully-pipelined chunk streaming.  Hidden streams are staged in
512-column L-chunks (host-packed, one contiguous 16KB-per-partition line
per chunk) on the gpsimd software-DGE queue, which sustains ~400GB/s; the
small constants ride the sync queue in first-use order (wv/wk jump the
x-queue since they gate the first matmuls).  Per chunk c the emission is

  [outproj(h1,c-1) l-tiles spread in] proj_v(c) proj_k(c)+rope
  proj_q0(c)+rope  attn(h0,c){proj_q1(c) spread in}  attn(h1,c){outproj
  (h0,c) spread in}

so the PE always has ready work: score matmuls of softmax pair p+1 are
emitted before the attnV/rowsum matmuls of pair p (hiding the ACT exp),
and projection/output-projection matmuls fill any exp-latency slack.
Output stores issue from the sync queue per l-tile; the final l-tiles
store in 512-column pieces to shorten the drain tail.

PSUM (8 banks): 3x(128,1024) "big" pool (two rotating score pairs + one
accumulator tile: pv|pk, pq0, or pattn|psums), 2x(128,512) transient pool
(pq1, rotate-half, pdiag, outproj tiles).
"""

import os
import sys

sys.path.insert(0, "/opt/trn_rl_repo")

import numpy as np

import concourse.bass as bass
import concourse.tile as tile
from concourse import bacc, mybir
from concourse.bass_utils import run_bass_kernel_spmd

H, HKV, D, HD, L = 16, 4, 2048, 128, 2048
THETA = 10000.0
NC = 8
HPC = H // NC  # q heads per core (2)
NDT = D // 128  # d-tiles (16)
NLT = L // 128  # l/j tiles (16)
NCH = L // 512  # 512-wide chunks (4)
F16 = mybir.dt.float16
F32 = mybir.dt.float32
EXP = mybir.ActivationFunctionType.Exp
CPY = mybir.ActivationFunctionType.Copy
MASKV = -30000.0  # additive mask; exp(x-30000) == 0 in fp16

last_exec_time_ns = None
last_mean_exec_time_ns = None

_programs = {}


def _build_program(causal: bool) -> bass.Bass:
    nc = bacc.Bacc(None, target_bir_lowering=False)

    xq = nc.dram_tensor("xq", [HPC, NCH, 128, NDT, 512], F16, kind="ExternalInput")
    xk = nc.dram_tensor("xk", [NCH, 128, NDT, 512], F16, kind="ExternalInput")
    xv = nc.dram_tensor("xv", [NCH, 128, NDT, 512], F16, kind="ExternalInput")
    wq = nc.dram_tensor("wq", [128, HPC, NDT, 128], F16, kind="ExternalInput")
    wk = nc.dram_tensor("wk", [128, NDT, 128], F16, kind="ExternalInput")
    wv = nc.dram_tensor("wv", [128, NDT, 128], F16, kind="ExternalInput")
    wo = nc.dram_tensor("wo", [128, HPC, D], F16, kind="ExternalInput")
    cosq = nc.dram_tensor("cosq", [128, L], F16, kind="ExternalInput")
    sinq = nc.dram_tensor("sinq", [128, L], F16, kind="ExternalInput")
    cosk = nc.dram_tensor("cosk", [128, L], F16, kind="ExternalInput")
    sink = nc.dram_tensor("sink", [128, L], F16, kind="ExternalInput")
    # misc fp16 constants: [:, :128] rotate-half perm (lhsT), [:, 128:256] ones
    misc = nc.dram_tensor("misc", [128, 256], F16, kind="ExternalInput")
    e1 = nc.dram_tensor("e1", [128, 1], F32, kind="ExternalInput")
    if causal:
        mask4 = nc.dram_tensor("mask4", [128, 2, 1024], F16, kind="ExternalInput")
        bmask = nc.dram_tensor("bmask", [128, 2, 1024], F16, kind="ExternalInput")
    else:
        maskg = nc.dram_tensor(
            "maskg", [128, NLT // 2, NCH, 1024], F32, kind="ExternalInput"
        )
    out = nc.dram_tensor("out", [HPC, NLT, 128, D], F16, kind="ExternalOutput")

    NSUB = 4  # x-chunk sub-DMAs (4 d-tiles each) for finer arrival granularity

    with tile.TileContext(nc) as tc:
        with (
            tc.tile_pool(name="const", bufs=1) as constp,
            tc.tile_pool(name="xs", bufs=20) as xpool,
            tc.tile_pool(name="persist", bufs=1) as persist,
            tc.tile_pool(name="probs", bufs=4) as pepool,
            tc.tile_pool(name="small", bufs=4) as smallp,
            tc.tile_pool(name="stage", bufs=4) as stagep,
            tc.tile_pool(name="outs", bufs=4) as outsp,
            tc.tile_pool(name="recs", bufs=4) as recs,
            # PSUM: 2*2 + 1*2 + 2*1 = 8 banks.  The acc pool holds the one
            # long-lived accumulator of each phase (pv|pk, pq0, pattn|psums);
            # score pairs rotate through sppool; everything short-lived
            # (pq1, rotate-half, pdiag, outproj) rotates through transp.
            tc.tile_pool(name="spbig", bufs=2, space="PSUM") as sppool,
            tc.tile_pool(name="accb", bufs=1, space="PSUM") as accpool,
            tc.tile_pool(name="trans", bufs=2, space="PSUM") as transp,
        ):
            # ---- wv/wk gate the first matmuls: load them on the x queue ----
            wv_sb = constp.tile([128, NDT, 128], F16, tag="wv")
            nc.gpsimd.dma_start(out=wv_sb[:], in_=wv[:])
            # remaining constants on the sync queue, in first-use order
            misc_sb = constp.tile([128, 256], F16, tag="misc")
            nc.sync.dma_start(out=misc_sb[:], in_=misc[:])
            w_all = constp.tile([128, HPC, NDT, 128], F16, tag="wq")
            nc.sync.dma_start(out=w_all[:], in_=wq[:])
            cosq_sb = constp.tile([128, L], F16, tag="cosq")
            nc.sync.dma_start(out=cosq_sb[:], in_=cosq[:])
            sinq_sb = constp.tile([128, L], F16, tag="sinq")
            nc.sync.dma_start(out=sinq_sb[:], in_=sinq[:])
            cosk_sb = constp.tile([128, L], F16, tag="cosk")
            nc.sync.dma_start(out=cosk_sb[:], in_=cosk[:])
            sink_sb = constp.tile([128, L], F16, tag="sink")
            nc.sync.dma_start(out=sink_sb[:], in_=sink[:])
            if causal:
                mask_sb = constp.tile([128, 2, 1024], F16, tag="mask")
                nc.sync.dma_start(out=mask_sb[:], in_=mask4[:])
                bmask_sb = constp.tile([128, 2, 1024], F16, tag="bmask")
                nc.sync.dma_start(out=bmask_sb[:], in_=bmask[:])
            e1_sb = constp.tile([128, 1], F32, tag="e1")
            nc.sync.dma_start(out=e1_sb[:], in_=e1[:])
            wo_sb = constp.tile([128, HPC, D], F16, tag="wo")
            nc.sync.dma_start(out=wo_sb[:], in_=wo[:])
            perm = misc_sb[:, 0:128]
            ones = misc_sb[:, 128:256]

            # persistent per-core activations
            krot = persist.tile([128, L], F16, tag="krot")
            v16 = persist.tile([128, L], F16, tag="v16")
            qcur = [
                persist.tile([128, 512], F16, tag=f"qcur{h}", name=f"qcur{h}")
                for h in range(HPC)
            ]
            wk_sb = constp.tile([128, NDT, 128], F16, tag="wk")

            def load_chunk(x_dram, c):
                xts = []
                step = NDT // NSUB
                for s in range(NSUB):
                    xt = xpool.tile([128, step, 512], F16, tag="xt")
                    nc.gpsimd.dma_start(
                        out=xt[:], in_=x_dram[c, :, s * step : (s + 1) * step, :]
                    )
                    xts.append(xt)
                return xts

            def proj_v(c):
                """v16 tiles for l-tiles 4c..4c+3; the q0 projection shares
                the second bank of the same accumulator tile (pv | pq0)."""
                xts = load_chunk(xv, c)
                pvk = accpool.tile([128, 1024], F32, tag="accb")
                step = NDT // NSUB
                for dt in range(NDT):
                    xt = xts[dt // step]
                    for ll in range(4):
                        nc.tensor.matmul(
                            pvk[:, ll * 128 : (ll + 1) * 128],
                            xt[:, dt % step, ll * 128 : (ll + 1) * 128],
                            wv_sb[:, dt, :],
                            start=(dt == 0 and ll == 0),
                            stop=(dt == NDT - 1 and ll == 3),
                            skip_group_check=True,
                        )
                nc.vector.tensor_copy(
                    out=v16[:, c * 512 : (c + 1) * 512], in_=pvk[:, 0:512]
                )
                return pvk

            def proj_kq_mms(c, x_dram, w_sb, preg, head=None):
                """Emit the 16 accumulation matmuls of one K/Q chunk into
                psum region preg; returns nothing (rope applied separately)."""
                xts = load_chunk(x_dram, c) if head is None else load_chunk(
                    x_dram[head], c
                )
                step = NDT // NSUB
                for dt in range(NDT):
                    xt = xts[dt // step]
                    nc.tensor.matmul(
                        preg,
                        w_sb[:, dt, :],
                        xt[:, dt % step, :],
                        start=(dt == 0),
                        stop=(dt == NDT - 1),
                        skip_group_check=True,
                    )

            def rope(c, preg, cos_sb, sin_sb, dst_sl):
                # t0 is the last reader of preg: emit it first so the psum
                # accumulator frees as early as possible; fp16 ops at 2x rate
                sl = slice(c * 512, (c + 1) * 512)
                u16 = smallp.tile([128, 512], F16, tag="u16")
                nc.scalar.copy(out=u16[:], in_=preg)
                t0 = smallp.tile([128, 512], F16, tag="ropetmp")
                nc.vector.tensor_mul(out=t0[:], in0=preg, in1=cos_sb[:, sl])
                rh = transp.tile([128, 512], F32, tag="trans")
                nc.tensor.matmul(rh[:], perm, u16[:])
                nc.vector.tensor_mul(out=dst_sl, in0=rh[:], in1=sin_sb[:, sl])
                nc.vector.tensor_add(out=dst_sl, in0=dst_sl, in1=t0[:])

            def attn_parts(i, c):
                npairs = 2 * (c + 1) if causal else NLT // 2
                pacc = accpool.tile([128, 1024], F32, tag="accb")
                pattn = pacc[:, 0:512]
                psums = pacc[:, 512:1024]
                # late chunks: DMA prefetch runs a chunk ahead, so the gpsimd
                # queue engine is free — offload the softmax row-sum
                # accumulation and mask application to it (saves PE matmuls
                # and DVE adds in the region where both are near-saturated)
                gp = causal and c >= 2
                accsum = (
                    stagep.tile([128, 512], F16, tag="sums32",
                                name=f"accsum{i}_{c}")
                    if gp else None
                )

                def emit_scores(p):
                    jt0 = 2 * p
                    if causal and jt0 >= 4 * c:
                        r0 = jt0 - 4 * c
                        lo0, lo1 = 128 * r0, 128 * (r0 + 1)
                        diag = True
                    else:
                        r0 = 0
                        lo0 = lo1 = 0
                        diag = False
                    sp = sppool.tile([128, 1024], F32, tag="spbig")
                    nc.tensor.matmul(
                        sp[:, lo0:512],
                        krot[:, jt0 * 128 : (jt0 + 1) * 128],
                        qcur[i][:, lo0:512],
                    )
                    nc.tensor.matmul(
                        sp[:, 512 + lo1 : 1024],
                        krot[:, (jt0 + 1) * 128 : (jt0 + 2) * 128],
                        qcur[i][:, lo1:512],
                    )
                    if causal:
                        if diag and not gp:
                            nc.vector.tensor_add(
                                out=sp[:, lo0:1024],
                                in0=sp[:, lo0:1024],
                                in1=mask_sb[:, r0 // 2, lo0:1024],
                            )
                    else:
                        mg = smallp.tile([128, 1024], F32, tag="maskg")
                        nc.gpsimd.dma_start(out=mg[:], in_=maskg[:, p, c, :])
                        nc.vector.tensor_add(out=sp[:], in0=sp[:], in1=mg[:])
                    pe = pepool.tile([128, 1024], F16, tag="probs")
                    nc.scalar.activation(
                        out=pe[:, lo0:1024], in_=sp[:, lo0:1024], func=EXP
                    )
                    if gp and diag:
                        # binary mask multiply on gpsimd replaces the additive
                        # mask (zeros in pe's masked/garbage columns)
                        nc.gpsimd.tensor_mul(
                            out=pe[:, lo0:1024],
                            in0=pe[:, lo0:1024],
                            in1=bmask_sb[:, r0 // 2, lo0:1024],
                        )
                    return (jt0, lo0, lo1, pe)

                def emit_av(state, first, last):
                    jt0, lo0, lo1, pe = state
                    nc.tensor.matmul(
                        pattn[:, lo0:512],
                        v16[:, jt0 * 128 : (jt0 + 1) * 128],
                        pe[:, lo0:512],
                        start=first, stop=False, skip_group_check=True,
                    )
                    nc.tensor.matmul(
                        pattn[:, lo1:512],
                        v16[:, (jt0 + 1) * 128 : (jt0 + 2) * 128],
                        pe[:, 512 + lo1 : 1024],
                        start=False, stop=last, skip_group_check=True,
                    )
                    if gp:
                        if first:
                            nc.gpsimd.tensor_copy(
                                out=accsum[:, lo0:512], in_=pe[:, lo0:512]
                            )
                        else:
                            nc.gpsimd.tensor_add(
                                out=accsum[:, lo0:512],
                                in0=accsum[:, lo0:512],
                                in1=pe[:, lo0:512],
                            )
                        nc.gpsimd.tensor_add(
                            out=accsum[:, lo1:512],
                            in0=accsum[:, lo1:512],
                            in1=pe[:, 512 + lo1 : 1024],
                        )
                    else:
                        nc.tensor.matmul(
                            psums[:, lo0:512], ones, pe[:, lo0:512],
                            start=first, stop=False, skip_group_check=True,
                        )
                        nc.tensor.matmul(
                            psums[:, lo1:512], ones, pe[:, 512 + lo1 : 1024],
                            start=False, stop=last, skip_group_check=True,
                        )

                def finish():
                    if gp:
                        # accsum already holds per-(j_local, l) partial sums in
                        # SBUF; pdiag's e1-matmul contracts the partition dim
                        sums32 = accsum
                    else:
                        sums32 = stagep.tile([128, 512], F32, tag="sums32")
                        nc.vector.tensor_copy(out=sums32[:], in_=psums[:])
                    attn16 = stagep.tile([128, 512], F16, tag="attn16")
                    nc.scalar.copy(out=attn16[:], in_=pattn[:])
                    return sums32, attn16, pacc

                return npairs, emit_scores, emit_av, finish

            def run_attn(i, c, interleave=None):
                npairs, emit_scores, emit_av, finish = attn_parts(i, c)
                inter = list(interleave) if interleave else []
                slots = {}
                if inter:
                    for k, w in enumerate(inter):
                        pos = 1 + (k * max(npairs - 1, 1)) // len(inter)
                        slots.setdefault(min(pos, npairs - 1), []).append(w)
                prev = None
                for p in range(npairs):
                    st = emit_scores(p)
                    if prev is not None:
                        emit_av(prev, first=(p == 1), last=False)
                    for w in slots.get(p, []):
                        w()
                    prev = st
                emit_av(prev, first=(npairs == 1), last=True)
                return finish()

            def outproj_tiles(i, c, sums32, attn16, pacc, fine=False,
                              store_gp=False):
                # pdiag reuses the (drained) psums bank of this head's pacc.
                # PE path: sums32 rows are broadcast totals -> e1 picks row 0.
                # gpsimd path: sums32 rows are per-j_local partials -> the
                # ones column completes the partition reduction.
                gp = causal and c >= 2
                rvec = ones[:, 0:1] if gp else e1_sb[:]
                pdiag = pacc[:, 512:516]
                for ls in range(4):
                    nc.tensor.matmul(
                        pdiag[:, ls : ls + 1],
                        sums32[:, ls * 128 : (ls + 1) * 128],
                        rvec,
                        skip_group_check=True,
                    )
                recip = recs.tile([128, 4], F32, tag="recip")
                nc.vector.reciprocal(out=recip[:], in_=pdiag[:])

                def make(ls):
                    def emit():
                        lt = 4 * c + ls
                        a_sl = attn16[:, ls * 128 : (ls + 1) * 128]
                        r_sl = recip[:, ls : ls + 1]
                        ost = outsp.tile([128, D], F16, tag="ost")
                        for dq in range(4):
                            po = transp.tile([128, 512], F32, tag="trans")
                            nc.tensor.matmul(
                                po[:], a_sl,
                                wo_sb[:, i, dq * 512 : (dq + 1) * 512],
                            )
                            osl = ost[:, dq * 512 : (dq + 1) * 512]
                            if fine:
                                # tail: split each drain across both engines
                                nc.vector.tensor_scalar_mul(
                                    out=ost[:, dq * 512 : dq * 512 + 256],
                                    in0=po[:, 0:256], scalar1=r_sl,
                                )
                                nc.scalar.activation(
                                    out=ost[:, dq * 512 + 256 : (dq + 1) * 512],
                                    in_=po[:, 256:512], func=CPY, scale=r_sl,
                                )
                            elif (ls * 4 + dq) % 8 in (1, 4, 6):
                                # ~5:3 DVE:ACT split balances both engines
                                nc.scalar.activation(
                                    out=osl, in_=po[:], func=CPY, scale=r_sl
                                )
                            else:
                                nc.vector.tensor_scalar_mul(
                                    out=osl, in0=po[:], scalar1=r_sl
                                )
                        # full-l-tile stores (4KB partition lines); the last
                        # chunk fans out across queues so the drain tail is
                        # not gated by a single ~250GB/s store ring
                        if fine:
                            eng = [nc.gpsimd, nc.sync, nc.scalar, nc.gpsimd][ls]
                        elif store_gp:
                            eng = nc.gpsimd if ls % 2 == 0 else nc.sync
                        else:
                            eng = nc.sync
                        eng.dma_start(out=out[i, lt], in_=ost[:])

                    return emit

                return [make(ls) for ls in range(4)]

            # ---- main pipeline over chunks ----
            # stream order per chunk: v, q0, k.  RoPE of q0 runs during the
            # k-projection matmuls; for c>=1 the first attention pairs touch
            # only old krot tiles, so rope_k latency is off the critical path.
            pending_h1 = None
            for c in range(NCH):
                pend = list(pending_h1) if pending_h1 else []
                pending_h1 = None
                pvk = proj_v(c)
                proj_kq_mms(c, xq, w_all[:, 0], pvk[:, 512:1024], head=0)
                rope(c, pvk[:, 512:1024], cosq_sb, sinq_sb, qcur[0][:])
                if c == 0:
                    nc.gpsimd.dma_start(out=wk_sb[:], in_=wk[:])
                pk = transp.tile([128, 512], F32, tag="trans", name=f"pk_{c}")
                proj_kq_mms(c, xk, wk_sb, pk[:])
                rope(c, pk[:], cosk_sb, sink_sb,
                     krot[:, c * 512 : (c + 1) * 512])

                # attn(h0): interleave q1 projection groups first (so rope_q1
                # lands well before attn(h1)), then h1's previous outproj
                pq1 = [None]

                def q1_group(g):
                    def emit():
                        if g == 0:
                            pq1[0] = transp.tile(
                                [128, 512], F32, tag="trans", name=f"pq1_{c}"
                            )
                            xts = load_chunk(xq[1], c)
                            pq1.append(xts)
                        xts = pq1[1]
                        step = NDT // NSUB
                        for dt in range(4 * g, 4 * g + 4):
                            nc.tensor.matmul(
                                pq1[0][:],
                                w_all[:, 1, dt, :],
                                xts[dt // step][:, dt % step, :],
                                start=(dt == 0),
                                stop=(dt == NDT - 1),
                                skip_group_check=True,
                            )
                        if g == 3:
                            rope(c, pq1[0][:], cosq_sb, sinq_sb, qcur[1][:])

                    return emit

                inter0 = [q1_group(g) for g in range(4)] + pend
                s0, a0, pc0 = run_attn(0, c, interleave=inter0)
                op0 = outproj_tiles(0, c, s0, a0, pc0,
                                    store_gp=(c == NCH - 1))
                s1, a1, pc1 = run_attn(1, c, interleave=op0)
                pending_h1 = outproj_tiles(1, c, s1, a1, pc1,
                                           fine=(c == NCH - 1))

            for w in pending_h1:
                w()
    nc.compile()
    return nc


def _get_program(causal: bool) -> bass.Bass:
    if causal not in _programs:
        _programs[causal] = _build_program(causal)
    return _programs[causal]


def _rope_tables(position_ids: np.ndarray):
    pos = position_ids.reshape(-1).astype(np.float32)  # (L,)
    inv_freq = (
        1.0 / (THETA ** (np.arange(0, HD, 2, dtype=np.float32) / HD))
    ).astype(np.float32)
    freqs = pos[:, None] * inv_freq[None, :]  # (L, HD/2)
    emb = np.concatenate([freqs, freqs], axis=1)  # (L, HD)
    cos = np.cos(emb).T.astype(np.float32).copy()  # (HD, L)
    sin = np.sin(emb).T.astype(np.float32).copy()
    return cos, sin


def _xt_chunks(x):  # (L, D) fp32 -> (NCH, 128, NDT, 512) fp16 chunked lhsT tiles
    xt = x.T.astype(np.float16).reshape(NDT, 128, NCH, 512)
    return np.ascontiguousarray(xt.transpose(2, 1, 0, 3))


def kernel(
    q_hidden, k_hidden, v_hidden, wq, wk, wv, wo, attention_mask, position_ids
):
    global last_exec_time_ns, last_mean_exec_time_ns
    q_hidden = np.asarray(q_hidden)
    k_hidden = np.asarray(k_hidden)
    v_hidden = np.asarray(v_hidden)
    wq = np.asarray(wq, dtype=np.float32)
    wk = np.asarray(wk, dtype=np.float32)
    wv = np.asarray(wv, dtype=np.float32)
    wo = np.asarray(wo, dtype=np.float32)
    attention_mask = np.asarray(attention_mask, dtype=np.float32)
    position_ids = np.asarray(position_ids)

    mask2d = attention_mask.reshape(L, L)
    causal_ref = np.where(
        np.tril(np.ones((L, L), dtype=bool)), np.float32(0.0), np.float32(-1e9)
    )
    causal = bool(np.array_equal(mask2d, causal_ref))

    cos, sin = _rope_tables(position_ids)
    scale = np.float32(1.0 / np.sqrt(HD))
    cosq_h = (cos * scale).astype(np.float16)
    sinq_h = (sin * scale).astype(np.float16)
    cosk_h = cos.astype(np.float16)
    sink_h = sin.astype(np.float16)

    # diagonal-band causal mask tiles: allowed iff j_local + 128*r <= l_local,
    # packed as j-tile pairs: pair q holds r=2q | r=2q+1 side by side
    jj = np.arange(128, dtype=np.int32)[:, None]
    ll = np.arange(512, dtype=np.int32)[None, :]
    _mr = [
        np.where(jj + 128 * r <= ll, np.float16(0.0), np.float16(MASKV))
        for r in range(4)
    ]
    mask4_h = np.stack(
        [np.concatenate([_mr[0], _mr[1]], axis=1),
         np.concatenate([_mr[2], _mr[3]], axis=1)],
        axis=1,
    ).astype(np.float16)  # (128, 2, 1024)
    bmask_h = (mask4_h == np.float16(0.0)).astype(np.float16)

    misc_h = np.zeros((128, 256), dtype=np.float16)
    # rotate-half: rh = P @ q with P[i, i+64] = -1 (i<64), P[i, i-64] = +1;
    # stored as lhsT = P^T
    for a in range(64):
        misc_h[a, a + 64] = np.float16(1.0)
        misc_h[a + 64, a] = np.float16(-1.0)
    misc_h[:, 128:256] = np.float16(1.0)  # ones block
    e1_h = np.zeros((128, 1), dtype=np.float32)
    e1_h[0, 0] = 1.0

    wq_r = wq.reshape(H, HD, D)
    wk_r = wk.reshape(HKV, HD, D)
    wv_r = wv.reshape(HKV, HD, D)
    wo_r = wo.reshape(D, H, HD)

    if not causal:
        mt = mask2d.T.reshape(NLT, 128, NCH, 512)
        maskg_h = np.ascontiguousarray(
            np.concatenate([mt[0::2], mt[1::2]], axis=3).transpose(1, 0, 2, 3)
        ).astype(np.float32)

    in_maps = []
    for core in range(NC):
        heads = [HPC * core + i for i in range(HPC)]
        g = heads[0] // (H // HKV)
        wq_t = np.stack(
            [
                wq_r[n].T.astype(np.float16).reshape(NDT, 128, HD)
                for n in heads
            ],
            axis=0,
        )  # (HPC, NDT, 128p, 128m)
        wq_t = np.ascontiguousarray(wq_t.transpose(2, 0, 1, 3))
        wk_t = wk_r[g].T.astype(np.float16).reshape(NDT, 128, HD)
        wk_t = np.ascontiguousarray(wk_t.transpose(1, 0, 2))
        wv_t = wv_r[g].T.astype(np.float16).reshape(NDT, 128, HD)
        wv_t = np.ascontiguousarray(wv_t.transpose(1, 0, 2))
        wo_t = np.stack(
            [wo_r[:, n, :].T.astype(np.float16) for n in heads], axis=0
        )  # (HPC, 128, D)
        wo_t = np.ascontiguousarray(wo_t.transpose(1, 0, 2))

        m = {
            "xq": np.stack([_xt_chunks(q_hidden[n, 0]) for n in heads], axis=0),
            "xk": _xt_chunks(k_hidden[g, 0]),
            "xv": _xt_chunks(v_hidden[g, 0]),
            "wq": wq_t,
            "wk": wk_t,
            "wv": wv_t,
            "wo": wo_t,
            "cosq": cosq_h,
            "sinq": sinq_h,
            "cosk": cosk_h,
            "sink": sink_h,
            "misc": misc_h,
            "e1": e1_h,
        }
        if causal:
            m["mask4"] = mask4_h
            m["bmask"] = bmask_h
        else:
            m["maskg"] = maskg_h
        in_maps.append(m)

    nc = _get_program(causal)
    trace_env = os.environ.get("KERNEL_TRACE", "0")
    kwargs = {}
    if trace_env != "0":
        kwargs["trace"] = True
        if trace_env == "8":
            kwargs["trace_cores"] = list(range(NC))
    res = run_bass_kernel_spmd(nc, in_maps, core_ids=list(range(NC)), **kwargs)
    last_exec_time_ns = res.exec_time_ns
    last_mean_exec_time_ns = res.mean_exec_time_ns
    globals()["last_results"] = res.results
    globals()["last_in_maps"] = in_maps
    globals()["last_res"] = res

    out = np.empty((H, 1, L, D), dtype=np.float32)
    for core in range(NC):
        o = res.results[core]["out"]  # (HPC, NLT, 128, D) fp16
        for i in range(HPC):
            out[HPC * core + i, 0] = o[i].reshape(L, D).astype(np.float32)
    return out


# revision 19
# speedup vs baseline: 1.0741x; 1.0741x over previous
"""Trainium2 Bass kernel for per-head Llama GQA attention.

Model: H=16 q heads, HKV=4 kv heads, head_dim=128, L=2048, D=2048, B=1.
Per-head hidden streams and per-head outputs (no cross-head reduction), so
tensor-parallel over heads is embarrassingly parallel: core c owns q heads
{2c, 2c+1} and their kv head c//2.  No collectives.

v3: fully-pipelined chunk streaming.  Hidden streams are staged in
512-column L-chunks (host-packed, one contiguous 16KB-per-partition line
per chunk) on the gpsimd software-DGE queue, which sustains ~400GB/s; the
small constants ride the sync queue in first-use order (wv/wk jump the
x-queue since they gate the first matmuls).  Per chunk c the emission is

  [outproj(h1,c-1) l-tiles spread in] proj_v(c) proj_k(c)+rope
  proj_q0(c)+rope  attn(h0,c){proj_q1(c) spread in}  attn(h1,c){outproj
  (h0,c) spread in}

so the PE always has ready work: score matmuls of softmax pair p+1 are
emitted before the attnV/rowsum matmuls of pair p (hiding the ACT exp),
and projection/output-projection matmuls fill any exp-latency slack.
Output stores issue from the sync queue per l-tile; the final l-tiles
store in 512-column pieces to shorten the drain tail.

PSUM (8 banks): 3x(128,1024) "big" pool (two rotating score pairs + one
accumulator tile: pv|pk, pq0, or pattn|psums), 2x(128,512) transient pool
(pq1, rotate-half, pdiag, outproj tiles).
"""

import os
import sys

sys.path.insert(0, "/opt/trn_rl_repo")

import numpy as np

import concourse.bass as bass
import concourse.tile as tile
from concourse import bacc, mybir
from concourse.bass_utils import run_bass_kernel_spmd

H, HKV, D, HD, L = 16, 4, 2048, 128, 2048
THETA = 10000.0
NC = 8
HPC = H // NC  # q heads per core (2)
NDT = D // 128  # d-tiles (16)
NLT = L // 128  # l/j tiles (16)
NCH = L // 512  # 512-wide chunks (4)
F16 = mybir.dt.float16
F32 = mybir.dt.float32
EXP = mybir.ActivationFunctionType.Exp
CPY = mybir.ActivationFunctionType.Copy
MASKV = -30000.0  # additive mask; exp(x-30000) == 0 in fp16

last_exec_time_ns = None
last_mean_exec_time_ns = None

_programs = {}


def _build_program(causal: bool) -> bass.Bass:
    nc = bacc.Bacc(None, target_bir_lowering=False)

    xq = nc.dram_tensor("xq", [HPC, NCH, 128, NDT, 512], F16, kind="ExternalInput")
    xk = nc.dram_tensor("xk", [NCH, 128, NDT, 512], F16, kind="ExternalInput")
    xv = nc.dram_tensor("xv", [NCH, 128, NDT, 512], F16, kind="ExternalInput")
    wq = nc.dram_tensor("wq", [128, HPC, NDT, 128], F16, kind="ExternalInput")
    wk = nc.dram_tensor("wk", [128, NDT, 128], F16, kind="ExternalInput")
    wv = nc.dram_tensor("wv", [128, NDT, 128], F16, kind="ExternalInput")
    wo = nc.dram_tensor("wo", [128, HPC, D], F16, kind="ExternalInput")
    cosq = nc.dram_tensor("cosq", [128, L], F16, kind="ExternalInput")
    sinq = nc.dram_tensor("sinq", [128, L], F16, kind="ExternalInput")
    cosk = nc.dram_tensor("cosk", [128, L], F16, kind="ExternalInput")
    sink = nc.dram_tensor("sink", [128, L], F16, kind="ExternalInput")
    # misc fp16 constants: [:, :128] rotate-half perm (lhsT), [:, 128:256] ones
    misc = nc.dram_tensor("misc", [128, 256], F16, kind="ExternalInput")
    e1 = nc.dram_tensor("e1", [128, 1], F32, kind="ExternalInput")
    if causal:
        mask4 = nc.dram_tensor("mask4", [128, 2, 1024], F16, kind="ExternalInput")
        bmask = nc.dram_tensor("bmask", [128, 2, 1024], F16, kind="ExternalInput")
    else:
        maskg = nc.dram_tensor(
            "maskg", [128, NLT // 2, NCH, 1024], F32, kind="ExternalInput"
        )
    out = nc.dram_tensor("out", [HPC, NLT, 128, D], F16, kind="ExternalOutput")

    NSUB = 4  # x-chunk sub-DMAs (4 d-tiles each) for finer arrival granularity

    with tile.TileContext(nc) as tc:
        with (
            tc.tile_pool(name="const", bufs=1) as constp,
            tc.tile_pool(name="xs", bufs=20) as xpool,
            tc.tile_pool(name="persist", bufs=1) as persist,
            tc.tile_pool(name="probs", bufs=4) as pepool,
            tc.tile_pool(name="small", bufs=4) as smallp,
            tc.tile_pool(name="stage", bufs=4) as stagep,
            tc.tile_pool(name="outs", bufs=4) as outsp,
            tc.tile_pool(name="recs", bufs=4) as recs,
            # PSUM: 2*2 + 1*2 + 2*1 = 8 banks.  The acc pool holds the one
            # long-lived accumulator of each phase (pv|pk, pq0, pattn|psums);
            # score pairs rotate through sppool; everything short-lived
            # (pq1, rotate-half, pdiag, outproj) rotates through transp.
            tc.tile_pool(name="spbig", bufs=2, space="PSUM") as sppool,
            tc.tile_pool(name="accb", bufs=1, space="PSUM") as accpool,
            tc.tile_pool(name="trans", bufs=2, space="PSUM") as transp,
        ):
            # ---- wv/wk gate the first matmuls: load them on the x queue ----
            wv_sb = constp.tile([128, NDT, 128], F16, tag="wv")
            nc.gpsimd.dma_start(out=wv_sb[:], in_=wv[:])
            # remaining constants on the sync queue, in first-use order
            misc_sb = constp.tile([128, 256], F16, tag="misc")
            nc.sync.dma_start(out=misc_sb[:], in_=misc[:])
            w_all = constp.tile([128, HPC, NDT, 128], F16, tag="wq")
            nc.sync.dma_start(out=w_all[:], in_=wq[:])
            cosq_sb = constp.tile([128, L], F16, tag="cosq")
            nc.sync.dma_start(out=cosq_sb[:], in_=cosq[:])
            sinq_sb = constp.tile([128, L], F16, tag="sinq")
            nc.sync.dma_start(out=sinq_sb[:], in_=sinq[:])
            cosk_sb = constp.tile([128, L], F16, tag="cosk")
            nc.sync.dma_start(out=cosk_sb[:], in_=cosk[:])
            sink_sb = constp.tile([128, L], F16, tag="sink")
            nc.sync.dma_start(out=sink_sb[:], in_=sink[:])
            if causal:
                mask_sb = constp.tile([128, 2, 1024], F16, tag="mask")
                nc.sync.dma_start(out=mask_sb[:], in_=mask4[:])
                bmask_sb = constp.tile([128, 2, 1024], F16, tag="bmask")
                nc.sync.dma_start(out=bmask_sb[:], in_=bmask[:])
            e1_sb = constp.tile([128, 1], F32, tag="e1")
            nc.sync.dma_start(out=e1_sb[:], in_=e1[:])
            wo_sb = constp.tile([128, HPC, D], F16, tag="wo")
            nc.sync.dma_start(out=wo_sb[:], in_=wo[:])
            perm = misc_sb[:, 0:128]
            ones = misc_sb[:, 128:256]

            # persistent per-core activations
            krot = persist.tile([128, L], F16, tag="krot")
            v16 = persist.tile([128, L], F16, tag="v16")
            qcur = [
                persist.tile([128, 512], F16, tag=f"qcur{h}", name=f"qcur{h}")
                for h in range(HPC)
            ]
            wk_sb = constp.tile([128, NDT, 128], F16, tag="wk")

            def load_chunk(x_dram, c):
                xts = []
                step = NDT // NSUB
                for s in range(NSUB):
                    xt = xpool.tile([128, step, 512], F16, tag="xt")
                    nc.gpsimd.dma_start(
                        out=xt[:], in_=x_dram[c, :, s * step : (s + 1) * step, :]
                    )
                    xts.append(xt)
                return xts

            def proj_v(c):
                """v16 tiles for l-tiles 4c..4c+3; the q0 projection shares
                the second bank of the same accumulator tile (pv | pq0)."""
                xts = load_chunk(xv, c)
                pvk = accpool.tile([128, 1024], F32, tag="accb")
                step = NDT // NSUB
                for dt in range(NDT):
                    xt = xts[dt // step]
                    for ll in range(4):
                        nc.tensor.matmul(
                            pvk[:, ll * 128 : (ll + 1) * 128],
                            xt[:, dt % step, ll * 128 : (ll + 1) * 128],
                            wv_sb[:, dt, :],
                            start=(dt == 0 and ll == 0),
                            stop=(dt == NDT - 1 and ll == 3),
                            skip_group_check=True,
                        )
                nc.vector.tensor_copy(
                    out=v16[:, c * 512 : (c + 1) * 512], in_=pvk[:, 0:512]
                )
                return pvk

            def proj_kq_mms(c, x_dram, w_sb, preg, head=None):
                """Emit the 16 accumulation matmuls of one K/Q chunk into
                psum region preg; returns nothing (rope applied separately)."""
                xts = load_chunk(x_dram, c) if head is None else load_chunk(
                    x_dram[head], c
                )
                step = NDT // NSUB
                for dt in range(NDT):
                    xt = xts[dt // step]
                    nc.tensor.matmul(
                        preg,
                        w_sb[:, dt, :],
                        xt[:, dt % step, :],
                        start=(dt == 0),
                        stop=(dt == NDT - 1),
                        skip_group_check=True,
                    )

            def rope(c, preg, cos_sb, sin_sb, dst_sl):
                # t0 is the last reader of preg: emit it first so the psum
                # accumulator frees as early as possible; fp16 ops at 2x rate
                sl = slice(c * 512, (c + 1) * 512)
                u16 = smallp.tile([128, 512], F16, tag="u16")
                nc.scalar.copy(out=u16[:], in_=preg)
                t0 = smallp.tile([128, 512], F16, tag="ropetmp")
                nc.vector.tensor_mul(out=t0[:], in0=preg, in1=cos_sb[:, sl])
                rh = transp.tile([128, 512], F32, tag="trans")
                nc.tensor.matmul(rh[:], perm, u16[:])
                nc.vector.tensor_mul(out=dst_sl, in0=rh[:], in1=sin_sb[:, sl])
                nc.vector.tensor_add(out=dst_sl, in0=dst_sl, in1=t0[:])

            def attn_parts(i, c):
                npairs = 2 * (c + 1) if causal else NLT // 2
                pacc = accpool.tile([128, 1024], F32, tag="accb")
                pattn = pacc[:, 0:512]
                psums = pacc[:, 512:1024]
                # late chunks: DMA prefetch runs a chunk ahead, so the gpsimd
                # queue engine is free — offload the softmax row-sum
                # accumulation and mask application to it (saves PE matmuls
                # and DVE adds in the region where both are near-saturated)
                # NOTE: offloading row-sums/mask to the Pool engine was
                # tried and reverted — Pool elementwise throughput is ~4x
                # slower than DVE in practice and it stalls the PE.
                gp = False
                accsum = (
                    stagep.tile([128, 512], F16, tag="sums32",
                                name=f"accsum{i}_{c}")
                    if gp else None
                )

                def emit_scores(p):
                    jt0 = 2 * p
                    if causal and jt0 >= 4 * c:
                        r0 = jt0 - 4 * c
                        lo0, lo1 = 128 * r0, 128 * (r0 + 1)
                        diag = True
                    else:
                        r0 = 0
                        lo0 = lo1 = 0
                        diag = False
                    sp = sppool.tile([128, 1024], F32, tag="spbig")
                    nc.tensor.matmul(
                        sp[:, lo0:512],
                        krot[:, jt0 * 128 : (jt0 + 1) * 128],
                        qcur[i][:, lo0:512],
                    )
                    nc.tensor.matmul(
                        sp[:, 512 + lo1 : 1024],
                        krot[:, (jt0 + 1) * 128 : (jt0 + 2) * 128],
                        qcur[i][:, lo1:512],
                    )
                    if causal:
                        if diag and not gp:
                            nc.vector.tensor_add(
                                out=sp[:, lo0:1024],
                                in0=sp[:, lo0:1024],
                                in1=mask_sb[:, r0 // 2, lo0:1024],
                            )
                    else:
                        mg = smallp.tile([128, 1024], F32, tag="maskg")
                        nc.gpsimd.dma_start(out=mg[:], in_=maskg[:, p, c, :])
                        nc.vector.tensor_add(out=sp[:], in0=sp[:], in1=mg[:])
                    pe = pepool.tile([128, 1024], F16, tag="probs")
                    nc.scalar.activation(
                        out=pe[:, lo0:1024], in_=sp[:, lo0:1024], func=EXP
                    )
                    if gp and diag:
                        # binary mask multiply on gpsimd replaces the additive
                        # mask (zeros in pe's masked/garbage columns)
                        nc.gpsimd.tensor_mul(
                            out=pe[:, lo0:1024],
                            in0=pe[:, lo0:1024],
                            in1=bmask_sb[:, r0 // 2, lo0:1024],
                        )
                    return (jt0, lo0, lo1, pe)

                def emit_av(state, first, last):
                    jt0, lo0, lo1, pe = state
                    nc.tensor.matmul(
                        pattn[:, lo0:512],
                        v16[:, jt0 * 128 : (jt0 + 1) * 128],
                        pe[:, lo0:512],
                        start=first, stop=False, skip_group_check=True,
                    )
                    nc.tensor.matmul(
                        pattn[:, lo1:512],
                        v16[:, (jt0 + 1) * 128 : (jt0 + 2) * 128],
                        pe[:, 512 + lo1 : 1024],
                        start=False, stop=last, skip_group_check=True,
                    )
                    if gp:
                        if first:
                            nc.gpsimd.tensor_copy(
                                out=accsum[:, lo0:512], in_=pe[:, lo0:512]
                            )
                        else:
                            nc.gpsimd.tensor_add(
                                out=accsum[:, lo0:512],
                                in0=accsum[:, lo0:512],
                                in1=pe[:, lo0:512],
                            )
                        nc.gpsimd.tensor_add(
                            out=accsum[:, lo1:512],
                            in0=accsum[:, lo1:512],
                            in1=pe[:, 512 + lo1 : 1024],
                        )
                    else:
                        nc.tensor.matmul(
                            psums[:, lo0:512], ones, pe[:, lo0:512],
                            start=first, stop=False, skip_group_check=True,
                        )
                        nc.tensor.matmul(
                            psums[:, lo1:512], ones, pe[:, 512 + lo1 : 1024],
                            start=False, stop=last, skip_group_check=True,
                        )

                def finish():
                    if gp:
                        # accsum already holds per-(j_local, l) partial sums in
                        # SBUF; pdiag's e1-matmul contracts the partition dim
                        sums32 = accsum
                    else:
                        sums32 = stagep.tile([128, 512], F32, tag="sums32")
                        nc.vector.tensor_copy(out=sums32[:], in_=psums[:])
                    attn16 = stagep.tile([128, 512], F16, tag="attn16")
                    nc.scalar.copy(out=attn16[:], in_=pattn[:])
                    return sums32, attn16, pacc

                return npairs, emit_scores, emit_av, finish

            def run_attn(i, c, interleave=None):
                npairs, emit_scores, emit_av, finish = attn_parts(i, c)
                inter = list(interleave) if interleave else []
                slots = {}
                if inter:
                    for k, w in enumerate(inter):
                        pos = 1 + (k * max(npairs - 1, 1)) // len(inter)
                        slots.setdefault(min(pos, npairs - 1), []).append(w)
                prev = None
                for p in range(npairs):
                    st = emit_scores(p)
                    if prev is not None:
                        emit_av(prev, first=(p == 1), last=False)
                    for w in slots.get(p, []):
                        w()
                    prev = st
                emit_av(prev, first=(npairs == 1), last=True)
                return finish()

            def outproj_tiles(i, c, sums32, attn16, pacc, fine=False,
                              store_gp=False):
                # pdiag reuses the (drained) psums bank of this head's pacc.
                # PE path: sums32 rows are broadcast totals -> e1 picks row 0.
                # gpsimd path: sums32 rows are per-j_local partials -> the
                # ones column completes the partition reduction.
                gp = False
                rvec = ones[:, 0:1] if gp else e1_sb[:]
                pdiag = pacc[:, 512:516]
                for ls in range(4):
                    nc.tensor.matmul(
                        pdiag[:, ls : ls + 1],
                        sums32[:, ls * 128 : (ls + 1) * 128],
                        rvec,
                        skip_group_check=True,
                    )
                recip = recs.tile([128, 4], F32, tag="recip")
                nc.vector.reciprocal(out=recip[:], in_=pdiag[:])

                def make(ls):
                    def emit():
                        lt = 4 * c + ls
                        a_sl = attn16[:, ls * 128 : (ls + 1) * 128]
                        r_sl = recip[:, ls : ls + 1]
                        ost = outsp.tile([128, D], F16, tag="ost")
                        for dq in range(4):
                            po = transp.tile([128, 512], F32, tag="trans")
                            nc.tensor.matmul(
                                po[:], a_sl,
                                wo_sb[:, i, dq * 512 : (dq + 1) * 512],
                            )
                            osl = ost[:, dq * 512 : (dq + 1) * 512]
                            if fine:
                                # tail: split each drain across both engines
                                nc.vector.tensor_scalar_mul(
                                    out=ost[:, dq * 512 : dq * 512 + 256],
                                    in0=po[:, 0:256], scalar1=r_sl,
                                )
                                nc.scalar.activation(
                                    out=ost[:, dq * 512 + 256 : (dq + 1) * 512],
                                    in_=po[:, 256:512], func=CPY, scale=r_sl,
                                )
                            elif (ls * 4 + dq) % 8 in (1, 4, 6):
                                # ~5:3 DVE:ACT split balances both engines
                                nc.scalar.activation(
                                    out=osl, in_=po[:], func=CPY, scale=r_sl
                                )
                            else:
                                nc.vector.tensor_scalar_mul(
                                    out=osl, in0=po[:], scalar1=r_sl
                                )
                        # full-l-tile stores (4KB partition lines); the last
                        # chunk fans out across queues so the drain tail is
                        # not gated by a single ~250GB/s store ring
                        if fine:
                            eng = [nc.gpsimd, nc.sync, nc.scalar, nc.gpsimd][ls]
                        elif store_gp:
                            eng = nc.gpsimd if ls % 2 == 0 else nc.sync
                        else:
                            eng = nc.sync
                        eng.dma_start(out=out[i, lt], in_=ost[:])

                    return emit

                return [make(ls) for ls in range(4)]

            # ---- main pipeline over chunks ----
            # stream order per chunk: v, q0, k.  RoPE of q0 runs during the
            # k-projection matmuls; for c>=1 the first attention pairs touch
            # only old krot tiles, so rope_k latency is off the critical path.
            pending_h1 = None
            for c in range(NCH):
                pend = list(pending_h1) if pending_h1 else []
                pending_h1 = None
                pvk = proj_v(c)
                proj_kq_mms(c, xq, w_all[:, 0], pvk[:, 512:1024], head=0)
                rope(c, pvk[:, 512:1024], cosq_sb, sinq_sb, qcur[0][:])
                if c == 0:
                    nc.gpsimd.dma_start(out=wk_sb[:], in_=wk[:])
                pk = transp.tile([128, 512], F32, tag="trans", name=f"pk_{c}")
                proj_kq_mms(c, xk, wk_sb, pk[:])
                rope(c, pk[:], cosk_sb, sink_sb,
                     krot[:, c * 512 : (c + 1) * 512])

                # attn(h0): interleave q1 projection groups first (so rope_q1
                # lands well before attn(h1)), then h1's previous outproj
                pq1 = [None]

                def q1_group(g):
                    def emit():
                        if g == 0:
                            pq1[0] = transp.tile(
                                [128, 512], F32, tag="trans", name=f"pq1_{c}"
                            )
                            xts = load_chunk(xq[1], c)
                            pq1.append(xts)
                        xts = pq1[1]
                        step = NDT // NSUB
                        for dt in range(4 * g, 4 * g + 4):
                            nc.tensor.matmul(
                                pq1[0][:],
                                w_all[:, 1, dt, :],
                                xts[dt // step][:, dt % step, :],
                                start=(dt == 0),
                                stop=(dt == NDT - 1),
                                skip_group_check=True,
                            )
                        if g == 3:
                            rope(c, pq1[0][:], cosq_sb, sinq_sb, qcur[1][:])

                    return emit

                inter0 = [q1_group(g) for g in range(4)] + pend
                s0, a0, pc0 = run_attn(0, c, interleave=inter0)
                op0 = outproj_tiles(0, c, s0, a0, pc0,
                                    store_gp=(c == NCH - 1))
                s1, a1, pc1 = run_attn(1, c, interleave=op0)
                pending_h1 = outproj_tiles(1, c, s1, a1, pc1,
                                           fine=(c == NCH - 1))

            for w in pending_h1:
                w()
    nc.compile()
    return nc


def _get_program(causal: bool) -> bass.Bass:
    if causal not in _programs:
        _programs[causal] = _build_program(causal)
    return _programs[causal]


def _rope_tables(position_ids: np.ndarray):
    pos = position_ids.reshape(-1).astype(np.float32)  # (L,)
    inv_freq = (
        1.0 / (THETA ** (np.arange(0, HD, 2, dtype=np.float32) / HD))
    ).astype(np.float32)
    freqs = pos[:, None] * inv_freq[None, :]  # (L, HD/2)
    emb = np.concatenate([freqs, freqs], axis=1)  # (L, HD)
    cos = np.cos(emb).T.astype(np.float32).copy()  # (HD, L)
    sin = np.sin(emb).T.astype(np.float32).copy()
    return cos, sin


def _xt_chunks(x):  # (L, D) fp32 -> (NCH, 128, NDT, 512) fp16 chunked lhsT tiles
    xt = x.T.astype(np.float16).reshape(NDT, 128, NCH, 512)
    return np.ascontiguousarray(xt.transpose(2, 1, 0, 3))


def kernel(
    q_hidden, k_hidden, v_hidden, wq, wk, wv, wo, attention_mask, position_ids
):
    global last_exec_time_ns, last_mean_exec_time_ns
    q_hidden = np.asarray(q_hidden)
    k_hidden = np.asarray(k_hidden)
    v_hidden = np.asarray(v_hidden)
    wq = np.asarray(wq, dtype=np.float32)
    wk = np.asarray(wk, dtype=np.float32)
    wv = np.asarray(wv, dtype=np.float32)
    wo = np.asarray(wo, dtype=np.float32)
    attention_mask = np.asarray(attention_mask, dtype=np.float32)
    position_ids = np.asarray(position_ids)

    mask2d = attention_mask.reshape(L, L)
    causal_ref = np.where(
        np.tril(np.ones((L, L), dtype=bool)), np.float32(0.0), np.float32(-1e9)
    )
    causal = bool(np.array_equal(mask2d, causal_ref))

    cos, sin = _rope_tables(position_ids)
    scale = np.float32(1.0 / np.sqrt(HD))
    cosq_h = (cos * scale).astype(np.float16)
    sinq_h = (sin * scale).astype(np.float16)
    cosk_h = cos.astype(np.float16)
    sink_h = sin.astype(np.float16)

    # diagonal-band causal mask tiles: allowed iff j_local + 128*r <= l_local,
    # packed as j-tile pairs: pair q holds r=2q | r=2q+1 side by side
    jj = np.arange(128, dtype=np.int32)[:, None]
    ll = np.arange(512, dtype=np.int32)[None, :]
    _mr = [
        np.where(jj + 128 * r <= ll, np.float16(0.0), np.float16(MASKV))
        for r in range(4)
    ]
    mask4_h = np.stack(
        [np.concatenate([_mr[0], _mr[1]], axis=1),
         np.concatenate([_mr[2], _mr[3]], axis=1)],
        axis=1,
    ).astype(np.float16)  # (128, 2, 1024)
    bmask_h = (mask4_h == np.float16(0.0)).astype(np.float16)

    misc_h = np.zeros((128, 256), dtype=np.float16)
    # rotate-half: rh = P @ q with P[i, i+64] = -1 (i<64), P[i, i-64] = +1;
    # stored as lhsT = P^T
    for a in range(64):
        misc_h[a, a + 64] = np.float16(1.0)
        misc_h[a + 64, a] = np.float16(-1.0)
    misc_h[:, 128:256] = np.float16(1.0)  # ones block
    e1_h = np.zeros((128, 1), dtype=np.float32)
    e1_h[0, 0] = 1.0

    wq_r = wq.reshape(H, HD, D)
    wk_r = wk.reshape(HKV, HD, D)
    wv_r = wv.reshape(HKV, HD, D)
    wo_r = wo.reshape(D, H, HD)

    if not causal:
        mt = mask2d.T.reshape(NLT, 128, NCH, 512)
        maskg_h = np.ascontiguousarray(
            np.concatenate([mt[0::2], mt[1::2]], axis=3).transpose(1, 0, 2, 3)
        ).astype(np.float32)

    in_maps = []
    for core in range(NC):
        heads = [HPC * core + i for i in range(HPC)]
        g = heads[0] // (H // HKV)
        wq_t = np.stack(
            [
                wq_r[n].T.astype(np.float16).reshape(NDT, 128, HD)
                for n in heads
            ],
            axis=0,
        )  # (HPC, NDT, 128p, 128m)
        wq_t = np.ascontiguousarray(wq_t.transpose(2, 0, 1, 3))
        wk_t = wk_r[g].T.astype(np.float16).reshape(NDT, 128, HD)
        wk_t = np.ascontiguousarray(wk_t.transpose(1, 0, 2))
        wv_t = wv_r[g].T.astype(np.float16).reshape(NDT, 128, HD)
        wv_t = np.ascontiguousarray(wv_t.transpose(1, 0, 2))
        wo_t = np.stack(
            [wo_r[:, n, :].T.astype(np.float16) for n in heads], axis=0
        )  # (HPC, 128, D)
        wo_t = np.ascontiguousarray(wo_t.transpose(1, 0, 2))

        m = {
            "xq": np.stack([_xt_chunks(q_hidden[n, 0]) for n in heads], axis=0),
            "xk": _xt_chunks(k_hidden[g, 0]),
            "xv": _xt_chunks(v_hidden[g, 0]),
            "wq": wq_t,
            "wk": wk_t,
            "wv": wv_t,
            "wo": wo_t,
            "cosq": cosq_h,
            "sinq": sinq_h,
            "cosk": cosk_h,
            "sink": sink_h,
            "misc": misc_h,
            "e1": e1_h,
        }
        if causal:
            m["mask4"] = mask4_h
            m["bmask"] = bmask_h
        else:
            m["maskg"] = maskg_h
        in_maps.append(m)

    nc = _get_program(causal)
    trace_env = os.environ.get("KERNEL_TRACE", "0")
    kwargs = {}
    if trace_env != "0":
        kwargs["trace"] = True
        if trace_env == "8":
            kwargs["trace_cores"] = list(range(NC))
    res = run_bass_kernel_spmd(nc, in_maps, core_ids=list(range(NC)), **kwargs)
    last_exec_time_ns = res.exec_time_ns
    last_mean_exec_time_ns = res.mean_exec_time_ns
    globals()["last_results"] = res.results
    globals()["last_in_maps"] = in_maps
    globals()["last_res"] = res

    out = np.empty((H, 1, L, D), dtype=np.float32)
    for core in range(NC):
        o = res.results[core]["out"]  # (HPC, NLT, 128, D) fp16
        for i in range(HPC):
            out[HPC * core + i, 0] = o[i].reshape(L, D).astype(np.float32)
    return out


# revision 20
# speedup vs baseline: 1.1346x; 1.0563x over previous
"""Trainium2 Bass kernel for per-head Llama GQA attention.

Model: H=16 q heads, HKV=4 kv heads, head_dim=128, L=2048, D=2048, B=1.
Per-head hidden streams and per-head outputs (no cross-head reduction), so
tensor-parallel over heads is embarrassingly parallel: core c owns q heads
{2c, 2c+1} and their kv head c//2.  No collectives.

v3: fully-pipelined chunk streaming.  Hidden streams are staged in
512-column L-chunks (host-packed, one contiguous 16KB-per-partition line
per chunk) on the gpsimd software-DGE queue, which sustains ~400GB/s; the
small constants ride the sync queue in first-use order (wv/wk jump the
x-queue since they gate the first matmuls).  Per chunk c the emission is

  [outproj(h1,c-1) l-tiles spread in] proj_v(c) proj_k(c)+rope
  proj_q0(c)+rope  attn(h0,c){proj_q1(c) spread in}  attn(h1,c){outproj
  (h0,c) spread in}

so the PE always has ready work: score matmuls of softmax pair p+1 are
emitted before the attnV/rowsum matmuls of pair p (hiding the ACT exp),
and projection/output-projection matmuls fill any exp-latency slack.
Output stores issue from the sync queue per l-tile; the final l-tiles
store in 512-column pieces to shorten the drain tail.

PSUM (8 banks): 3x(128,1024) "big" pool (two rotating score pairs + one
accumulator tile: pv|pk, pq0, or pattn|psums), 2x(128,512) transient pool
(pq1, rotate-half, pdiag, outproj tiles).
"""

import os
import sys

sys.path.insert(0, "/opt/trn_rl_repo")

import numpy as np

import concourse.bass as bass
import concourse.tile as tile
from concourse import bacc, mybir
from concourse.bass_utils import run_bass_kernel_spmd

H, HKV, D, HD, L = 16, 4, 2048, 128, 2048
THETA = 10000.0
NC = 8
HPC = H // NC  # q heads per core (2)
NDT = D // 128  # d-tiles (16)
NLT = L // 128  # l/j tiles (16)
NCH = L // 512  # 512-wide chunks (4)
F16 = mybir.dt.float16
F32 = mybir.dt.float32
EXP = mybir.ActivationFunctionType.Exp
CPY = mybir.ActivationFunctionType.Copy
MASKV = -30000.0  # additive mask; exp(x-30000) == 0 in fp16

last_exec_time_ns = None
last_mean_exec_time_ns = None

_programs = {}


def _build_program(causal: bool) -> bass.Bass:
    nc = bacc.Bacc(None, target_bir_lowering=False)

    xq = nc.dram_tensor("xq", [HPC, NCH, 128, NDT, 512], F16, kind="ExternalInput")
    xk = nc.dram_tensor("xk", [NCH, 128, NDT, 512], F16, kind="ExternalInput")
    xv = nc.dram_tensor("xv", [NCH, 128, NDT, 512], F16, kind="ExternalInput")
    wq = nc.dram_tensor("wq", [128, HPC, NDT, 128], F16, kind="ExternalInput")
    wk = nc.dram_tensor("wk", [128, NDT, 128], F16, kind="ExternalInput")
    wv = nc.dram_tensor("wv", [128, NDT, 128], F16, kind="ExternalInput")
    wo = nc.dram_tensor("wo", [128, HPC, D], F16, kind="ExternalInput")
    cosq = nc.dram_tensor("cosq", [128, L], F16, kind="ExternalInput")
    sinq = nc.dram_tensor("sinq", [128, L], F16, kind="ExternalInput")
    cosk = nc.dram_tensor("cosk", [128, L], F16, kind="ExternalInput")
    sink = nc.dram_tensor("sink", [128, L], F16, kind="ExternalInput")
    # misc fp16 constants: [:, :128] rotate-half perm (lhsT), [:, 128:256] ones
    misc = nc.dram_tensor("misc", [128, 256], F16, kind="ExternalInput")
    e1 = nc.dram_tensor("e1", [128, 1], F32, kind="ExternalInput")
    if causal:
        mask4 = nc.dram_tensor("mask4", [128, 2, 1024], F16, kind="ExternalInput")
    else:
        maskg = nc.dram_tensor(
            "maskg", [128, NLT // 2, NCH, 1024], F32, kind="ExternalInput"
        )
    out = nc.dram_tensor("out", [HPC, NLT, 128, D], F16, kind="ExternalOutput")

    NSUB = 4  # x-chunk sub-DMAs (4 d-tiles each) for finer arrival granularity

    with tile.TileContext(nc) as tc:
        with (
            tc.tile_pool(name="const", bufs=1) as constp,
            tc.tile_pool(name="xs", bufs=20) as xpool,
            tc.tile_pool(name="persist", bufs=1) as persist,
            tc.tile_pool(name="probs", bufs=4) as pepool,
            tc.tile_pool(name="small", bufs=4) as smallp,
            tc.tile_pool(name="stage", bufs=4) as stagep,
            tc.tile_pool(name="outs", bufs=4) as outsp,
            tc.tile_pool(name="recs", bufs=4) as recs,
            # PSUM: 2*2 + 1*2 + 2*1 = 8 banks.  The acc pool holds the one
            # long-lived accumulator of each phase (pv|pk, pq0, pattn|psums);
            # score pairs rotate through sppool; everything short-lived
            # (pq1, rotate-half, pdiag, outproj) rotates through transp.
            tc.tile_pool(name="spbig", bufs=2, space="PSUM") as sppool,
            tc.tile_pool(name="accb", bufs=1, space="PSUM") as accpool,
            tc.tile_pool(name="trans", bufs=2, space="PSUM") as transp,
        ):
            # ---- wv/wk gate the first matmuls: load them on the x queue ----
            wv_sb = constp.tile([128, NDT, 128], F16, tag="wv")
            nc.gpsimd.dma_start(out=wv_sb[:], in_=wv[:])
            # remaining constants on the sync queue, in first-use order
            misc_sb = constp.tile([128, 256], F16, tag="misc")
            nc.sync.dma_start(out=misc_sb[:], in_=misc[:])
            w_all = constp.tile([128, HPC, NDT, 128], F16, tag="wq")
            nc.sync.dma_start(out=w_all[:], in_=wq[:])
            # cos/sin tables are streamed per-chunk (see chunk loop) to
            # keep the fill-phase DMA window small
            cosq_sb = constp.tile([128, L], F16, tag="cosq")
            sinq_sb = constp.tile([128, L], F16, tag="sinq")
            cosk_sb = constp.tile([128, L], F16, tag="cosk")
            sink_sb = constp.tile([128, L], F16, tag="sink")
            if causal:
                mask_sb = constp.tile([128, 2, 1024], F16, tag="mask")
                nc.sync.dma_start(out=mask_sb[:], in_=mask4[:])
            e1_sb = constp.tile([128, 1], F32, tag="e1")
            nc.sync.dma_start(out=e1_sb[:], in_=e1[:])
            wo_sb = constp.tile([128, HPC, D], F16, tag="wo")
            nc.sync.dma_start(out=wo_sb[:], in_=wo[:])
            perm = misc_sb[:, 0:128]
            ones = misc_sb[:, 128:256]

            # persistent per-core activations
            krot = persist.tile([128, L], F16, tag="krot")
            v16 = persist.tile([128, L], F16, tag="v16")
            qcur = [
                persist.tile([128, 512], F16, tag=f"qcur{h}", name=f"qcur{h}")
                for h in range(HPC)
            ]
            wk_sb = constp.tile([128, NDT, 128], F16, tag="wk")

            def load_chunk(x_dram, c):
                xts = []
                step = NDT // NSUB
                for s in range(NSUB):
                    xt = xpool.tile([128, step, 512], F16, tag="xt")
                    nc.gpsimd.dma_start(
                        out=xt[:], in_=x_dram[c, :, s * step : (s + 1) * step, :]
                    )
                    xts.append(xt)
                return xts

            def proj_v(c):
                """v16 tiles for l-tiles 4c..4c+3; the q0 projection shares
                the second bank of the same accumulator tile (pv | pq0)."""
                xts = load_chunk(xv, c)
                pvk = accpool.tile([128, 1024], F32, tag="accb")
                step = NDT // NSUB
                for dt in range(NDT):
                    xt = xts[dt // step]
                    for ll in range(4):
                        nc.tensor.matmul(
                            pvk[:, ll * 128 : (ll + 1) * 128],
                            xt[:, dt % step, ll * 128 : (ll + 1) * 128],
                            wv_sb[:, dt, :],
                            start=(dt == 0 and ll == 0),
                            stop=(dt == NDT - 1 and ll == 3),
                            skip_group_check=True,
                        )
                nc.vector.tensor_copy(
                    out=v16[:, c * 512 : (c + 1) * 512], in_=pvk[:, 0:512]
                )
                return pvk

            def proj_kq_mms(c, x_dram, w_sb, preg, head=None):
                """Emit the 16 accumulation matmuls of one K/Q chunk into
                psum region preg; returns nothing (rope applied separately)."""
                xts = load_chunk(x_dram, c) if head is None else load_chunk(
                    x_dram[head], c
                )
                step = NDT // NSUB
                for dt in range(NDT):
                    xt = xts[dt // step]
                    nc.tensor.matmul(
                        preg,
                        w_sb[:, dt, :],
                        xt[:, dt % step, :],
                        start=(dt == 0),
                        stop=(dt == NDT - 1),
                        skip_group_check=True,
                    )

            def rope(c, preg, cos_sb, sin_sb, dst_sl):
                # t0 is the last reader of preg: emit it first so the psum
                # accumulator frees as early as possible; fp16 ops at 2x rate
                sl = slice(c * 512, (c + 1) * 512)
                u16 = smallp.tile([128, 512], F16, tag="u16")
                nc.scalar.copy(out=u16[:], in_=preg)
                t0 = smallp.tile([128, 512], F16, tag="ropetmp")
                nc.vector.tensor_mul(out=t0[:], in0=preg, in1=cos_sb[:, sl])
                rh = transp.tile([128, 512], F32, tag="trans")
                nc.tensor.matmul(rh[:], perm, u16[:])
                nc.vector.tensor_mul(out=dst_sl, in0=rh[:], in1=sin_sb[:, sl])
                nc.vector.tensor_add(out=dst_sl, in0=dst_sl, in1=t0[:])

            def attn_parts(i, c):
                npairs = 2 * (c + 1) if causal else NLT // 2
                pacc = accpool.tile([128, 1024], F32, tag="accb")
                pattn = pacc[:, 0:512]
                psums = pacc[:, 512:1024]
                # late chunks: DMA prefetch runs a chunk ahead, so the gpsimd
                # queue engine is free — offload the softmax row-sum
                # accumulation and mask application to it (saves PE matmuls
                # and DVE adds in the region where both are near-saturated)
                # NOTE: offloading row-sums/mask to the Pool engine was
                # tried and reverted — Pool elementwise throughput is ~4x
                # slower than DVE in practice and it stalls the PE.
                gp = False
                accsum = (
                    stagep.tile([128, 512], F16, tag="sums32",
                                name=f"accsum{i}_{c}")
                    if gp else None
                )

                def emit_scores(p):
                    jt0 = 2 * p
                    if causal and jt0 >= 4 * c:
                        r0 = jt0 - 4 * c
                        lo0, lo1 = 128 * r0, 128 * (r0 + 1)
                        diag = True
                    else:
                        r0 = 0
                        lo0 = lo1 = 0
                        diag = False
                    sp = sppool.tile([128, 1024], F32, tag="spbig")
                    nc.tensor.matmul(
                        sp[:, lo0:512],
                        krot[:, jt0 * 128 : (jt0 + 1) * 128],
                        qcur[i][:, lo0:512],
                    )
                    nc.tensor.matmul(
                        sp[:, 512 + lo1 : 1024],
                        krot[:, (jt0 + 1) * 128 : (jt0 + 2) * 128],
                        qcur[i][:, lo1:512],
                    )
                    if causal:
                        if diag and not gp:
                            nc.vector.tensor_add(
                                out=sp[:, lo0:1024],
                                in0=sp[:, lo0:1024],
                                in1=mask_sb[:, r0 // 2, lo0:1024],
                            )
                    else:
                        mg = smallp.tile([128, 1024], F32, tag="maskg")
                        nc.gpsimd.dma_start(out=mg[:], in_=maskg[:, p, c, :])
                        nc.vector.tensor_add(out=sp[:], in0=sp[:], in1=mg[:])
                    pe = pepool.tile([128, 1024], F16, tag="probs")
                    nc.scalar.activation(
                        out=pe[:, lo0:1024], in_=sp[:, lo0:1024], func=EXP
                    )
                    return (jt0, lo0, lo1, pe)

                def emit_av(state, first, last):
                    jt0, lo0, lo1, pe = state
                    nc.tensor.matmul(
                        pattn[:, lo0:512],
                        v16[:, jt0 * 128 : (jt0 + 1) * 128],
                        pe[:, lo0:512],
                        start=first, stop=False, skip_group_check=True,
                    )
                    nc.tensor.matmul(
                        pattn[:, lo1:512],
                        v16[:, (jt0 + 1) * 128 : (jt0 + 2) * 128],
                        pe[:, 512 + lo1 : 1024],
                        start=False, stop=last, skip_group_check=True,
                    )
                    if gp:
                        if first:
                            nc.gpsimd.tensor_copy(
                                out=accsum[:, lo0:512], in_=pe[:, lo0:512]
                            )
                        else:
                            nc.gpsimd.tensor_add(
                                out=accsum[:, lo0:512],
                                in0=accsum[:, lo0:512],
                                in1=pe[:, lo0:512],
                            )
                        nc.gpsimd.tensor_add(
                            out=accsum[:, lo1:512],
                            in0=accsum[:, lo1:512],
                            in1=pe[:, 512 + lo1 : 1024],
                        )
                    else:
                        nc.tensor.matmul(
                            psums[:, lo0:512], ones, pe[:, lo0:512],
                            start=first, stop=False, skip_group_check=True,
                        )
                        nc.tensor.matmul(
                            psums[:, lo1:512], ones, pe[:, 512 + lo1 : 1024],
                            start=False, stop=last, skip_group_check=True,
                        )

                def finish():
                    if gp:
                        # accsum already holds per-(j_local, l) partial sums in
                        # SBUF; pdiag's e1-matmul contracts the partition dim
                        sums32 = accsum
                    else:
                        sums32 = stagep.tile([128, 512], F32, tag="sums32")
                        nc.vector.tensor_copy(out=sums32[:], in_=psums[:])
                    attn16 = stagep.tile([128, 512], F16, tag="attn16")
                    nc.scalar.copy(out=attn16[:], in_=pattn[:])
                    return sums32, attn16, pacc

                return npairs, emit_scores, emit_av, finish

            def run_attn(i, c, interleave=None):
                npairs, emit_scores, emit_av, finish = attn_parts(i, c)
                inter = list(interleave) if interleave else []
                slots = {}
                if inter:
                    for k, w in enumerate(inter):
                        pos = 1 + (k * max(npairs - 1, 1)) // len(inter)
                        slots.setdefault(min(pos, npairs - 1), []).append(w)
                prev = None
                for p in range(npairs):
                    st = emit_scores(p)
                    if prev is not None:
                        emit_av(prev, first=(p == 1), last=False)
                    for w in slots.get(p, []):
                        w()
                    prev = st
                emit_av(prev, first=(npairs == 1), last=True)
                return finish()

            def outproj_tiles(i, c, sums32, attn16, pacc, fine=False,
                              store_gp=False):
                # pdiag reuses the (drained) psums bank of this head's pacc.
                # PE path: sums32 rows are broadcast totals -> e1 picks row 0.
                # gpsimd path: sums32 rows are per-j_local partials -> the
                # ones column completes the partition reduction.
                gp = False
                rvec = ones[:, 0:1] if gp else e1_sb[:]
                pdiag = pacc[:, 512:516]
                for ls in range(4):
                    nc.tensor.matmul(
                        pdiag[:, ls : ls + 1],
                        sums32[:, ls * 128 : (ls + 1) * 128],
                        rvec,
                        skip_group_check=True,
                    )
                recip = recs.tile([128, 4], F32, tag="recip")
                nc.vector.reciprocal(out=recip[:], in_=pdiag[:])

                def make(ls):
                    def emit():
                        lt = 4 * c + ls
                        a_sl = attn16[:, ls * 128 : (ls + 1) * 128]
                        r_sl = recip[:, ls : ls + 1]
                        ost = outsp.tile([128, D], F16, tag="ost")
                        for dq in range(4):
                            po = transp.tile([128, 512], F32, tag="trans")
                            nc.tensor.matmul(
                                po[:], a_sl,
                                wo_sb[:, i, dq * 512 : (dq + 1) * 512],
                            )
                            osl = ost[:, dq * 512 : (dq + 1) * 512]
                            if fine:
                                # tail: split each drain across both engines
                                nc.vector.tensor_scalar_mul(
                                    out=ost[:, dq * 512 : dq * 512 + 256],
                                    in0=po[:, 0:256], scalar1=r_sl,
                                )
                                nc.scalar.activation(
                                    out=ost[:, dq * 512 + 256 : (dq + 1) * 512],
                                    in_=po[:, 256:512], func=CPY, scale=r_sl,
                                )
                            elif (ls * 4 + dq) % 8 in (1, 4, 6):
                                # ~5:3 DVE:ACT split balances both engines
                                nc.scalar.activation(
                                    out=osl, in_=po[:], func=CPY, scale=r_sl
                                )
                            else:
                                nc.vector.tensor_scalar_mul(
                                    out=osl, in0=po[:], scalar1=r_sl
                                )
                        # full-l-tile stores (4KB partition lines); the last
                        # chunk fans out across queues so the drain tail is
                        # not gated by a single ~250GB/s store ring
                        if fine or store_gp:
                            eng = nc.gpsimd if ls % 2 == 0 else nc.sync
                        else:
                            eng = nc.sync
                        eng.dma_start(out=out[i, lt], in_=ost[:])

                    return emit

                return [make(ls) for ls in range(4)]

            # ---- main pipeline over chunks ----
            # stream order per chunk: v, q0, k.  RoPE of q0 runs during the
            # k-projection matmuls; for c>=1 the first attention pairs touch
            # only old krot tiles, so rope_k latency is off the critical path.
            pending_h1 = None
            for c in range(NCH):
                pend = list(pending_h1) if pending_h1 else []
                pending_h1 = None
                sl = slice(c * 512, (c + 1) * 512)
                nc.sync.dma_start(out=cosq_sb[:, sl], in_=cosq[:, sl])
                nc.sync.dma_start(out=sinq_sb[:, sl], in_=sinq[:, sl])
                nc.sync.dma_start(out=cosk_sb[:, sl], in_=cosk[:, sl])
                nc.sync.dma_start(out=sink_sb[:, sl], in_=sink[:, sl])
                pvk = proj_v(c)
                proj_kq_mms(c, xq, w_all[:, 0], pvk[:, 512:1024], head=0)
                rope(c, pvk[:, 512:1024], cosq_sb, sinq_sb, qcur[0][:])
                if c == 0:
                    nc.gpsimd.dma_start(out=wk_sb[:], in_=wk[:])
                pk = transp.tile([128, 512], F32, tag="trans", name=f"pk_{c}")
                proj_kq_mms(c, xk, wk_sb, pk[:])
                rope(c, pk[:], cosk_sb, sink_sb,
                     krot[:, c * 512 : (c + 1) * 512])

                # attn(h0): interleave q1 projection groups first (so rope_q1
                # lands well before attn(h1)), then h1's previous outproj
                pq1 = [None]

                def q1_group(g):
                    def emit():
                        if g == 0:
                            pq1[0] = transp.tile(
                                [128, 512], F32, tag="trans", name=f"pq1_{c}"
                            )
                            xts = load_chunk(xq[1], c)
                            pq1.append(xts)
                        xts = pq1[1]
                        step = NDT // NSUB
                        for dt in range(4 * g, 4 * g + 4):
                            nc.tensor.matmul(
                                pq1[0][:],
                                w_all[:, 1, dt, :],
                                xts[dt // step][:, dt % step, :],
                                start=(dt == 0),
                                stop=(dt == NDT - 1),
                                skip_group_check=True,
                            )
                        if g == 3:
                            rope(c, pq1[0][:], cosq_sb, sinq_sb, qcur[1][:])

                    return emit

                inter0 = [q1_group(g) for g in range(4)] + pend
                s0, a0, pc0 = run_attn(0, c, interleave=inter0)
                op0 = outproj_tiles(0, c, s0, a0, pc0,
                                    store_gp=(c == NCH - 1))
                s1, a1, pc1 = run_attn(1, c, interleave=op0)
                pending_h1 = outproj_tiles(1, c, s1, a1, pc1,
                                           fine=(c == NCH - 1),
                                           store_gp=(c == NCH - 2))

            for w in pending_h1:
                w()
    nc.compile()
    return nc


def _get_program(causal: bool) -> bass.Bass:
    if causal not in _programs:
        _programs[causal] = _build_program(causal)
    return _programs[causal]


def _rope_tables(position_ids: np.ndarray):
    pos = position_ids.reshape(-1).astype(np.float32)  # (L,)
    inv_freq = (
        1.0 / (THETA ** (np.arange(0, HD, 2, dtype=np.float32) / HD))
    ).astype(np.float32)
    freqs = pos[:, None] * inv_freq[None, :]  # (L, HD/2)
    emb = np.concatenate([freqs, freqs], axis=1)  # (L, HD)
    cos = np.cos(emb).T.astype(np.float32).copy()  # (HD, L)
    sin = np.sin(emb).T.astype(np.float32).copy()
    return cos, sin


def _xt_chunks(x):  # (L, D) fp32 -> (NCH, 128, NDT, 512) fp16 chunked lhsT tiles
    xt = x.T.astype(np.float16).reshape(NDT, 128, NCH, 512)
    return np.ascontiguousarray(xt.transpose(2, 1, 0, 3))


def kernel(
    q_hidden, k_hidden, v_hidden, wq, wk, wv, wo, attention_mask, position_ids
):
    global last_exec_time_ns, last_mean_exec_time_ns
    q_hidden = np.asarray(q_hidden)
    k_hidden = np.asarray(k_hidden)
    v_hidden = np.asarray(v_hidden)
    wq = np.asarray(wq, dtype=np.float32)
    wk = np.asarray(wk, dtype=np.float32)
    wv = np.asarray(wv, dtype=np.float32)
    wo = np.asarray(wo, dtype=np.float32)
    attention_mask = np.asarray(attention_mask, dtype=np.float32)
    position_ids = np.asarray(position_ids)

    mask2d = attention_mask.reshape(L, L)
    causal_ref = np.where(
        np.tril(np.ones((L, L), dtype=bool)), np.float32(0.0), np.float32(-1e9)
    )
    causal = bool(np.array_equal(mask2d, causal_ref))

    cos, sin = _rope_tables(position_ids)
    scale = np.float32(1.0 / np.sqrt(HD))
    cosq_h = (cos * scale).astype(np.float16)
    sinq_h = (sin * scale).astype(np.float16)
    cosk_h = cos.astype(np.float16)
    sink_h = sin.astype(np.float16)

    # diagonal-band causal mask tiles: allowed iff j_local + 128*r <= l_local,
    # packed as j-tile pairs: pair q holds r=2q | r=2q+1 side by side
    jj = np.arange(128, dtype=np.int32)[:, None]
    ll = np.arange(512, dtype=np.int32)[None, :]
    _mr = [
        np.where(jj + 128 * r <= ll, np.float16(0.0), np.float16(MASKV))
        for r in range(4)
    ]
    mask4_h = np.stack(
        [np.concatenate([_mr[0], _mr[1]], axis=1),
         np.concatenate([_mr[2], _mr[3]], axis=1)],
        axis=1,
    ).astype(np.float16)  # (128, 2, 1024)
    bmask_h = (mask4_h == np.float16(0.0)).astype(np.float16)

    misc_h = np.zeros((128, 256), dtype=np.float16)
    # rotate-half: rh = P @ q with P[i, i+64] = -1 (i<64), P[i, i-64] = +1;
    # stored as lhsT = P^T
    for a in range(64):
        misc_h[a, a + 64] = np.float16(1.0)
        misc_h[a + 64, a] = np.float16(-1.0)
    misc_h[:, 128:256] = np.float16(1.0)  # ones block
    e1_h = np.zeros((128, 1), dtype=np.float32)
    e1_h[0, 0] = 1.0

    wq_r = wq.reshape(H, HD, D)
    wk_r = wk.reshape(HKV, HD, D)
    wv_r = wv.reshape(HKV, HD, D)
    wo_r = wo.reshape(D, H, HD)

    if not causal:
        mt = mask2d.T.reshape(NLT, 128, NCH, 512)
        maskg_h = np.ascontiguousarray(
            np.concatenate([mt[0::2], mt[1::2]], axis=3).transpose(1, 0, 2, 3)
        ).astype(np.float32)

    in_maps = []
    for core in range(NC):
        heads = [HPC * core + i for i in range(HPC)]
        g = heads[0] // (H // HKV)
        wq_t = np.stack(
            [
                wq_r[n].T.astype(np.float16).reshape(NDT, 128, HD)
                for n in heads
            ],
            axis=0,
        )  # (HPC, NDT, 128p, 128m)
        wq_t = np.ascontiguousarray(wq_t.transpose(2, 0, 1, 3))
        wk_t = wk_r[g].T.astype(np.float16).reshape(NDT, 128, HD)
        wk_t = np.ascontiguousarray(wk_t.transpose(1, 0, 2))
        wv_t = wv_r[g].T.astype(np.float16).reshape(NDT, 128, HD)
        wv_t = np.ascontiguousarray(wv_t.transpose(1, 0, 2))
        wo_t = np.stack(
            [wo_r[:, n, :].T.astype(np.float16) for n in heads], axis=0
        )  # (HPC, 128, D)
        wo_t = np.ascontiguousarray(wo_t.transpose(1, 0, 2))

        m = {
            "xq": np.stack([_xt_chunks(q_hidden[n, 0]) for n in heads], axis=0),
            "xk": _xt_chunks(k_hidden[g, 0]),
            "xv": _xt_chunks(v_hidden[g, 0]),
            "wq": wq_t,
            "wk": wk_t,
            "wv": wv_t,
            "wo": wo_t,
            "cosq": cosq_h,
            "sinq": sinq_h,
            "cosk": cosk_h,
            "sink": sink_h,
            "misc": misc_h,
            "e1": e1_h,
        }
        if causal:
            m["mask4"] = mask4_h
        else:
            m["maskg"] = maskg_h
        in_maps.append(m)

    nc = _get_program(causal)
    trace_env = os.environ.get("KERNEL_TRACE", "0")
    kwargs = {}
    if trace_env != "0":
        kwargs["trace"] = True
        if trace_env == "8":
            kwargs["trace_cores"] = list(range(NC))
    res = run_bass_kernel_spmd(nc, in_maps, core_ids=list(range(NC)), **kwargs)
    last_exec_time_ns = res.exec_time_ns
    last_mean_exec_time_ns = res.mean_exec_time_ns
    globals()["last_results"] = res.results
    globals()["last_in_maps"] = in_maps
    globals()["last_res"] = res

    out = np.empty((H, 1, L, D), dtype=np.float32)
    for core in range(NC):
        o = res.results[core]["out"]  # (HPC, NLT, 128, D) fp16
        for i in range(HPC):
            out[HPC * core + i, 0] = o[i].reshape(L, D).astype(np.float32)
    return out
